# revision 6
# baseline (speedup 1.0000x reference)
"""Multi-head self-attention (B=16, N=784, D=768, H=12) on 8 trn2 cores.

Strategy: pure data-parallel over batch (2 batches per core, no collectives).
Host pre-casts x / weights / biases to bf16 (same rounding the device would
do); all matmuls run in bf16 with fp32 PSUM accumulation.

Per batch, on-device:
  A) X [784,768] is PE-transposed (bf16, 4 transposes packed per PSUM bank)
     into XT [768,784].
  B) QKV projection. Q,K are produced in transposed layout QKT [f, t]
     (stationary = Wqkv column block, moving = XT); the per-feature bias is
     added on DVE during the PSUM->SBUF copy. V is produced in natural
     layout [t, f] (stationary = XT chunk, moving = Wqkv V columns, bias via
     rank-1 accumulating matmul) and packed into an augmented slab
     [t, 12*(64+1)] whose per-head 65th column is 1.0.
  C) Per head: scores^T [tj, ti] = K^T-chunk.T @ Q^T (K=64 contraction);
     softmax without max-subtraction (scores are O(1) here): exp on ACT with
     the 1/8 scale fused in; PV with the ones-augmented V slab gives
     O^T[64,ti] plus the softmax denominator in row 64 for free. The PSUM
     result is copied out immediately (frees the bank), then normalized via
     reciprocal (DVE) + partition-broadcast + multiply on GpSimd.
  D) Output projection from OT (already the required lhsT layout) + bias via
     rank-1 accumulating matmuls.
"""

from contextlib import ExitStack

import ml_dtypes
import numpy as np

import concourse.bass as bass
import concourse.mybir as mybir
import concourse.tile as tile
from concourse import bacc
from concourse.bass_utils import run_bass_kernel_spmd
from concourse.masks import make_identity

dt = mybir.dt
AF = mybir.ActivationFunctionType

B, N, D = 16, 784, 768
H, HD = 12, 64
F3 = 3 * D  # 2304
N_CORES = 8
BPC = B // N_CORES  # batches per core

# token chunks: 784 = 6*128 + 16
T_CHUNKS = [(i * 128, min(128, N - i * 128)) for i in range((N + 127) // 128)]
NT = len(T_CHUNKS)  # 7
ND = D // 128  # 6 d-chunks
# transpose packing groups: 4+3 t-chunks -> one PSUM bank each
TR_GROUPS = [T_CHUNKS[0:4], T_CHUNKS[4:7]]
# psum column groups (bank-aligned: one fp32 bank holds 512)
COLS_N = [(0, 512), (512, N - 512)]   # over 784 tokens
COLS_D = [(0, 512), (512, D - 512)]   # over 768 features

BF = dt.bfloat16


def _setup_consts(nc, P, aps):
    ident = P["konst"].tile([128, 128], BF, name="ident")
    make_identity(nc, ident[:])
    ones_c = P["konst"].tile([1, N], BF, name="ones_c")
    nc.vector.memset(ones_c[:], 1.0)

    bqc = P["konst"].tile([128, F3 // 128], dt.float32, name="bqc")
    nc.sync.dma_start(bqc[:], aps["bqc"][:])
    bqv16 = P["konst"].tile([1, D], BF, name="bqv16")
    nc.sync.dma_start(bqv16[:], aps["bqv"][:])
    bo16 = P["konst"].tile([1, D], BF, name="bo16")
    nc.sync.dma_start(bo16[:], aps["bo"][:])

    wq16, wo16 = [], []
    for di in range(ND):
        w = P["wq"].tile([128, F3], BF, name=f"wq{di}", tag="wq")
        nc.sync.dma_start(w[:], aps["wqkv"][di * 128:(di + 1) * 128, :])
        wq16.append(w)
    for di in range(ND):
        w = P["wo"].tile([128, D], BF, name=f"wo{di}", tag="wo")
        nc.sync.dma_start(w[:], aps["wo"][di * 128:(di + 1) * 128, :])
        wo16.append(w)
    return dict(ident=ident, ones_c=ones_c, bqc=bqc, bqv16=bqv16, bo16=bo16,
                wq16=wq16, wo16=wo16)


def _phase_a(nc, P, C, aps, b):
    """Load X (bf16) and transpose to XT [768, 784]."""
    x16 = []
    for (t0, p), ti in zip(T_CHUNKS, range(NT)):
        x = P["xin"].tile([128, D], BF, name=f"x16_{b}_{ti}", tag="x16")
        nc.sync.dma_start(x[0:p, :], aps["xs"][b, t0:t0 + p, :])
        x16.append(x)
    xt16 = [P["xt"].tile([128, N], BF, name=f"xt{b}_{di}", tag="xt")
            for di in range(ND)]
    for di in range(ND):
        for grp in TR_GROUPS:
            g0 = grp[0][0]
            gw = grp[-1][0] + grp[-1][1] - g0
            tr = P["ps_tr"].tile([128, 512], BF, name="tr", tag="tr")
            for (t0, p) in grp:
                nc.tensor.transpose(
                    tr[0:128, t0 - g0:t0 - g0 + p],
                    x16[t0 // 128][0:p, di * 128:(di + 1) * 128],
                    C["ident"][0:p, 0:p])
            nc.vector.tensor_copy(xt16[di][:, g0:g0 + gw], tr[0:128, 0:gw])
    return xt16


def _phase_b_qk(nc, P, C, b, xt16):
    """Q,K in transposed layout: 12 tiles [128, 784]."""
    qkt16 = []
    for fi in range(12):  # Q: 0..5, K: 6..11
        qk_ps = P["ps_big"].tile([128, N], dt.float32, name="qk_ps", tag="big")
        for (c0, cw) in COLS_N:
            for di in range(ND):
                nc.tensor.matmul(
                    qk_ps[:, c0:c0 + cw],
                    C["wq16"][di][:, fi * 128:(fi + 1) * 128],
                    xt16[di][:, c0:c0 + cw],
                    start=(di == 0), stop=(di == ND - 1))
        q = P["qkt"].tile([128, N], BF, name=f"qkt{b}_{fi}", tag="qkt")
        nc.vector.tensor_scalar_add(q[:], qk_ps[:], C["bqc"][0:128, fi:fi + 1])
        qkt16.append(q)
    return qkt16


def _phase_b_v(nc, P, C, b, xt16):
    """V natural layout, packed per head with a trailing ones column."""
    vt16 = []
    for (t0, p), ti in zip(T_CHUNKS, range(NT)):
        v_ps = P["ps_big"].tile([128, N], dt.float32, name="v_ps", tag="big")
        for (c0, cw) in COLS_D:
            for di in range(ND):
                nc.tensor.matmul(
                    v_ps[0:p, c0:c0 + cw],
                    xt16[di][:, t0:t0 + p],
                    C["wq16"][di][:, 2 * D + c0:2 * D + c0 + cw],
                    start=(di == 0), stop=False)
            nc.tensor.matmul(
                v_ps[0:p, c0:c0 + cw],
                C["ones_c"][0:1, t0:t0 + p],
                C["bqv16"][0:1, c0:c0 + cw],
                start=False, stop=True)
        vt = P["vt"].tile([128, H, HD + 1], BF, name=f"vt{b}_{ti}", tag="vt")
        nc.vector.tensor_copy(vt[0:p, :, 0:HD],
                              v_ps[0:p, 0:D].rearrange("p (h d) -> p h d", h=H))
        nc.vector.memset(vt[0:p, :, HD:HD + 1], 1.0)
        vt16.append(vt)
    return vt16


def _head_scores(nc, P, qt, kt, ro):
    """scores^T -> exp, returns 7 expS^T tiles [tj, 784] bf16."""
    exl = []
    for (t0, pj), tj in zip(T_CHUNKS, range(NT)):
        sc_ps = P["ps_big"].tile([128, N], dt.float32, name="sc_ps", tag="big")
        for (c0, cw) in COLS_N:
            nc.tensor.matmul(
                sc_ps[0:pj, c0:c0 + cw],
                kt[ro:ro + HD, t0:t0 + pj],
                qt[ro:ro + HD, c0:c0 + cw],
                start=True, stop=True)
        ex = P["ex"].tile([128, N], BF, name="ex", tag="ex")
        nc.scalar.activation(ex[0:pj, :], sc_ps[0:pj, :], AF.Exp,
                             scale=float(HD) ** -0.5)
        exl.append(ex)
    return exl


def _head_pv(nc, P, h, vt16, exl):
    """PV with ones-augmented V: psum [65, 784]; row 64 = softmax denom."""
    ot_ps = P["ps_big"].tile([HD + 1, N], dt.float32, name="ot_ps", tag="big")
    for (c0, cw) in COLS_N:
        for (t0, pj), tj in zip(T_CHUNKS, range(NT)):
            nc.tensor.matmul(
                ot_ps[0:HD + 1, c0:c0 + cw],
                vt16[tj][0:pj, h, 0:HD + 1],
                exl[tj][0:pj, c0:c0 + cw],
                start=(tj == 0), stop=(tj == NT - 1))
    return ot_ps


def _phase_c(nc, P, b, qkt16, vt16):
    ot16 = [P["ot"].tile([128, N], BF, name=f"ot{b}_{oi}", tag="ot")
            for oi in range(ND)]
    for h in range(H):
        qt, kt, ro = qkt16[h // 2], qkt16[6 + h // 2], (h % 2) * HD
        exl = _head_scores(nc, P, qt, kt, ro)
        ot_ps = _head_pv(nc, P, h, vt16, exl)
        # copy out of PSUM immediately to free the bank, then normalize
        osb = P["osb"].tile([HD + 1, N], dt.float32, name="osb", tag="osb")
        nc.vector.tensor_copy(osb[:], ot_ps[:])
        rec = P["recp"].tile([1, N], dt.float32, name="rec", tag="rec")
        nc.vector.reciprocal(rec[0:1, :], osb[HD:HD + 1, :])
        brec = P["brec"].tile([HD, N], dt.float32, name="brec", tag="brec")
        nc.gpsimd.partition_broadcast(brec[0:HD, :], rec[0:1, :])
        nc.gpsimd.tensor_tensor(ot16[h // 2][ro:ro + HD, :],
                                osb[0:HD, :], brec[0:HD, :],
                                mybir.AluOpType.mult)
    return ot16


def _phase_d(nc, P, C, aps, b, ot16):
    for (t0, p), ti in zip(T_CHUNKS, range(NT)):
        y_ps = P["ps_big"].tile([128, N], dt.float32, name="y_ps", tag="big")
        for (c0, cw) in COLS_D:
            for oi in range(ND):
                nc.tensor.matmul(
                    y_ps[0:p, c0:c0 + cw],
                    ot16[oi][:, t0:t0 + p],
                    C["wo16"][oi][:, c0:c0 + cw],
                    start=(oi == 0), stop=False)
            nc.tensor.matmul(
                y_ps[0:p, c0:c0 + cw],
                C["ones_c"][0:1, t0:t0 + p],
                C["bo16"][0:1, c0:c0 + cw],
                start=False, stop=True)
        y32 = P["yout"].tile([128, D], dt.float32, name="y32", tag="y32")
        nc.vector.tensor_copy(y32[0:p, :], y_ps[0:p, 0:D])
        nc.sync.dma_start(aps["ys"][b, t0:t0 + p, :], y32[0:p, :])


POOL_SPECS = [
    ("konst", 1, "SBUF"), ("wq", ND, "SBUF"),
    ("wo", ND, "SBUF"), ("xin", NT, "SBUF"), ("xt", ND, "SBUF"),
    ("qkt", 12, "SBUF"), ("vt", NT, "SBUF"), ("ex", 14, "SBUF"),
    ("ot", ND, "SBUF"), ("osb", 3, "SBUF"), ("brec", 2, "SBUF"),
    ("recp", 2, "SBUF"), ("yout", 3, "SBUF"),
    ("ps_tr", 2, "PSUM"), ("ps_big", 3, "PSUM"),
]


def build():
    nc = bacc.Bacc("TRN2", target_bir_lowering=False, debug=False)

    aps = {
        "xs": nc.dram_tensor("xs", [BPC, N, D], BF, kind="ExternalInput").ap(),
        "wqkv": nc.dram_tensor("wqkv", [D, F3], BF, kind="ExternalInput").ap(),
        "bqc": nc.dram_tensor("bqc", [128, F3 // 128], dt.float32, kind="ExternalInput").ap(),
        "bqv": nc.dram_tensor("bqv", [1, D], BF, kind="ExternalInput").ap(),
        "wo": nc.dram_tensor("wo", [D, D], BF, kind="ExternalInput").ap(),
        "bo": nc.dram_tensor("bo", [1, D], BF, kind="ExternalInput").ap(),
        "ys": nc.dram_tensor("ys", [BPC, N, D], dt.float32, kind="ExternalOutput").ap(),
    }

    with ExitStack() as ctx:
        tc = ctx.enter_context(tile.TileContext(nc))
        P = {}
        for pname, bufs, space in POOL_SPECS:
            P[pname] = ctx.enter_context(
                tc.tile_pool(name=pname, bufs=bufs, space=space))

        C = _setup_consts(nc, P, aps)
        for b in range(BPC):
            xt16 = _phase_a(nc, P, C, aps, b)
            qkt16 = _phase_b_qk(nc, P, C, b, xt16)
            vt16 = _phase_b_v(nc, P, C, b, xt16)
            ot16 = _phase_c(nc, P, b, qkt16, vt16)
            _phase_d(nc, P, C, aps, b, ot16)

    nc.compile()
    return nc


_NC_CACHE = {}


def _get_nc():
    if "nc" not in _NC_CACHE:
        _NC_CACHE["nc"] = build()
    return _NC_CACHE["nc"]


def make_in_maps(x, Wqkv, bqkv, Wo, bo):
    bf = ml_dtypes.bfloat16
    x = np.asarray(x, dtype=np.float32)
    Wqkv16 = np.ascontiguousarray(np.asarray(Wqkv, np.float32).astype(bf))
    bqkv = np.asarray(bqkv, dtype=np.float32)
    Wo16 = np.ascontiguousarray(np.asarray(Wo, np.float32).astype(bf))
    bo = np.asarray(bo, dtype=np.float32)
    bqc = np.ascontiguousarray(bqkv.reshape(F3 // 128, 128).T)
    bqv = np.ascontiguousarray(bqkv[2 * D:].reshape(1, D).astype(bf))
    bo_r = np.ascontiguousarray(bo.reshape(1, D).astype(bf))
    x16 = x.astype(bf)
    in_maps = []
    for c in range(N_CORES):
        in_maps.append({
            "xs": np.ascontiguousarray(x16[c * BPC:(c + 1) * BPC]),
            "wqkv": Wqkv16,
            "bqc": bqc,
            "bqv": bqv,
            "wo": Wo16,
            "bo": bo_r,
        })
    return in_maps


def run(x, Wqkv, bqkv, Wo, bo, trace=False, **kw):
    nc = _get_nc()
    in_maps = make_in_maps(x, Wqkv, bqkv, Wo, bo)
    res = run_bass_kernel_spmd(nc, in_maps, list(range(N_CORES)), trace=trace, **kw)
    out = np.concatenate([res.results[c]["ys"] for c in range(N_CORES)], axis=0)
    return out, res


def kernel(x, Wqkv, bqkv, Wo, bo):
    out, _ = run(x, Wqkv, bqkv, Wo, bo)
    return out


# revision 8
# speedup vs baseline: 1.3678x; 1.3678x over previous
"""Multi-head self-attention (B=16, N=784, D=768, H=12) on 8 trn2 cores.

Strategy: pure data-parallel over batch (2 batches per core, no collectives).
Host pre-casts x / weights / biases to bf16 (same rounding the device would
do); all matmuls run in bf16 with fp32 PSUM accumulation.

Per batch, on-device:
  A) X [784,768] is PE-transposed (bf16, transposes packed 4-per-PSUM-bank)
     into XT [768,784].
  B) QKV projection. Q,K are produced in transposed layout QKT [f, t]
     (stationary = Wqkv column block, moving = XT); the per-feature bias is
     added on DVE during the PSUM->SBUF copy. V is produced in natural
     layout [t, f] (stationary = XT chunk, moving = Wqkv V columns, bias via
     rank-1 accumulating matmul) and packed into an augmented slab
     [t, 12*(64+1)] whose per-head 65th column is 1.0.
  C) Per head: scores^T [tj, ti] = K^T-chunk.T @ Q^T (K=64 contraction);
     softmax without max-subtraction (scores are O(1) here): exp on ACT with
     the 1/8 scale fused in; PV with the ones-augmented V slab gives
     O^T[64,ti] plus the softmax denominator in row 64 for free. The PSUM
     result is copied to SBUF immediately (frees the bank), then normalized:
     in-place reciprocal of the denominator row (DVE), partition-broadcast
     (GpSimd), multiply (DVE).
  D) Output projection from OT (already the required lhsT layout) + bias via
     rank-1 accumulating matmuls.

The two batches are software-pipelined at emission level: batch 1's
transpose/projection units are interleaved between batch 0's attention
heads, and batch 0's output projection is interleaved into batch 1's
attention, so the PE never drains while ACT (softmax exp) streams.
"""

from contextlib import ExitStack

import ml_dtypes
import numpy as np

import concourse.bass as bass
import concourse.mybir as mybir
import concourse.tile as tile
from concourse import bacc
from concourse.bass_utils import run_bass_kernel_spmd
from concourse.masks import make_identity

dt = mybir.dt
AF = mybir.ActivationFunctionType

B, N, D = 16, 784, 768
H, HD = 12, 64
F3 = 3 * D  # 2304
N_CORES = 8
BPC = B // N_CORES  # batches per core

# token chunks: 784 = 6*128 + 16
T_CHUNKS = [(i * 128, min(128, N - i * 128)) for i in range((N + 127) // 128)]
NT = len(T_CHUNKS)  # 7
ND = D // 128  # 6 d-chunks
# transpose packing groups: 4+3 t-chunks -> one PSUM bank each
TR_GROUPS = [T_CHUNKS[0:4], T_CHUNKS[4:7]]
# psum column groups (bank-aligned: one fp32 bank holds 512)
COLS_N = [(0, 512), (512, N - 512)]   # over 784 tokens
COLS_D = [(0, 512), (512, D - 512)]   # over 768 features

BF = dt.bfloat16


def _setup_consts(nc, P, aps):
    ident = P["konst"].tile([128, 128], BF, name="ident")
    make_identity(nc, ident[:])
    ones_c = P["konst"].tile([1, N], BF, name="ones_c")
    nc.vector.memset(ones_c[:], 1.0)

    bqc = P["konst"].tile([128, F3 // 128], dt.float32, name="bqc")
    nc.sync.dma_start(bqc[:], aps["bqc"][:])
    bqv16 = P["konst"].tile([1, D], BF, name="bqv16")
    nc.sync.dma_start(bqv16[:], aps["bqv"][:])
    bo16 = P["konst"].tile([1, D], BF, name="bo16")
    nc.sync.dma_start(bo16[:], aps["bo"][:])

    wq16, wo16 = [], []
    for di in range(ND):
        w = P["wq"].tile([128, F3], BF, name=f"wq{di}", tag="wq")
        nc.sync.dma_start(w[:], aps["wqkv"][di * 128:(di + 1) * 128, :])
        wq16.append(w)
    for di in range(ND):
        w = P["wo"].tile([128, D], BF, name=f"wo{di}", tag="wo")
        nc.sync.dma_start(w[:], aps["wo"][di * 128:(di + 1) * 128, :])
        wo16.append(w)
    return dict(ident=ident, ones_c=ones_c, bqc=bqc, bqv16=bqv16, bo16=bo16,
                wq16=wq16, wo16=wo16)


def _gen_a(nc, P, C, aps, b, st):
    """Load X (bf16) and transpose to XT [768, 784]. Yields per unit."""
    x16 = []
    for (t0, p), ti in zip(T_CHUNKS, range(NT)):
        x = P["xin"].tile([128, D], BF, name=f"x16_{b}_{ti}", tag="x16")
        nc.sync.dma_start(x[0:p, :], aps["xs"][b, t0:t0 + p, :])
        x16.append(x)
    yield
    xt16 = [P["xt"].tile([128, N], BF, name=f"xt{b}_{di}", tag="xt")
            for di in range(ND)]
    for di in range(ND):
        for grp in TR_GROUPS:
            g0 = grp[0][0]
            gw = grp[-1][0] + grp[-1][1] - g0
            tr = P["ps_big"].tile([128, 512], BF, name="tr", tag="big")
            for (t0, p) in grp:
                nc.tensor.transpose(
                    tr[0:128, t0 - g0:t0 - g0 + p],
                    x16[t0 // 128][0:p, di * 128:(di + 1) * 128],
                    C["ident"][0:p, 0:p])
            nc.vector.tensor_copy(xt16[di][:, g0:g0 + gw], tr[0:128, 0:gw])
        yield
    st[f"xt{b}"] = xt16


def _gen_b_qk(nc, P, C, b, st):
    """Q,K in transposed layout: 12 tiles [128, 784]. Yields per f-chunk."""
    xt16 = st[f"xt{b}"]
    qkt16 = []
    st[f"qkt{b}"] = qkt16
    for fi in range(12):  # Q: 0..5, K: 6..11
        qk_ps = P["ps_big"].tile([128, N], dt.float32, name="qk_ps", tag="big")
        for (c0, cw) in COLS_N:
            for di in range(ND):
                nc.tensor.matmul(
                    qk_ps[:, c0:c0 + cw],
                    C["wq16"][di][:, fi * 128:(fi + 1) * 128],
                    xt16[di][:, c0:c0 + cw],
                    start=(di == 0), stop=(di == ND - 1))
        q = P["qkt"].tile([128, N], BF, name=f"qkt{b}_{fi}", tag="qkt")
        nc.vector.tensor_scalar_add(q[:], qk_ps[:], C["bqc"][0:128, fi:fi + 1])
        qkt16.append(q)
        yield


def _gen_b_v(nc, P, C, b, st):
    """V natural layout + ones column. Yields per t-chunk."""
    xt16 = st[f"xt{b}"]
    vt16 = []
    st[f"vt{b}"] = vt16
    for (t0, p), ti in zip(T_CHUNKS, range(NT)):
        v_ps = P["ps_big"].tile([128, N], dt.float32, name="v_ps", tag="big")
        for (c0, cw) in COLS_D:
            for di in range(ND):
                nc.tensor.matmul(
                    v_ps[0:p, c0:c0 + cw],
                    xt16[di][:, t0:t0 + p],
                    C["wq16"][di][:, 2 * D + c0:2 * D + c0 + cw],
                    start=(di == 0), stop=False)
            nc.tensor.matmul(
                v_ps[0:p, c0:c0 + cw],
                C["ones_c"][0:1, t0:t0 + p],
                C["bqv16"][0:1, c0:c0 + cw],
                start=False, stop=True)
        vt = P["vt"].tile([128, H, HD + 1], BF, name=f"vt{b}_{ti}", tag="vt")
        nc.vector.tensor_copy(vt[0:p, :, 0:HD],
                              v_ps[0:p, 0:D].rearrange("p (h d) -> p h d", h=H))
        nc.vector.memset(vt[0:p, :, HD:HD + 1], 1.0)
        vt16.append(vt)
        yield


def _gen_c(nc, P, b, st):
    """Attention. Yields per head."""
    qkt16, vt16 = st[f"qkt{b}"], st[f"vt{b}"]
    ot16 = [P["ot"].tile([128, N], BF, name=f"ot{b}_{oi}", tag="ot")
            for oi in range(ND)]
    st[f"ot{b}"] = ot16
    for h in range(H):
        qt, kt, ro = qkt16[h // 2], qkt16[6 + h // 2], (h % 2) * HD
        # scores^T + exp per tj chunk
        exl = []
        for (t0, pj), tj in zip(T_CHUNKS, range(NT)):
            sc_ps = P["ps_big"].tile([128, N], dt.float32, name="sc_ps", tag="big")
            for (c0, cw) in COLS_N:
                nc.tensor.matmul(
                    sc_ps[0:pj, c0:c0 + cw],
                    kt[ro:ro + HD, t0:t0 + pj],
                    qt[ro:ro + HD, c0:c0 + cw],
                    start=True, stop=True)
            ex = P["ex"].tile([128, N], BF, name="ex", tag="ex")
            nc.scalar.activation(ex[0:pj, :], sc_ps[0:pj, :], AF.Exp,
                                 scale=float(HD) ** -0.5)
            exl.append(ex)
        # PV with ones-augmented V: row 64 = softmax denominator
        ot_ps = P["ps_big"].tile([HD + 1, N], dt.float32, name="ot_ps", tag="big")
        for (c0, cw) in COLS_N:
            for (t0, pj), tj in zip(T_CHUNKS, range(NT)):
                nc.tensor.matmul(
                    ot_ps[0:HD + 1, c0:c0 + cw],
                    vt16[tj][0:pj, h, 0:HD + 1],
                    exl[tj][0:pj, c0:c0 + cw],
                    start=(tj == 0), stop=(tj == NT - 1))
        # copy out of PSUM immediately (frees the bank), then normalize
        osb = P["osb"].tile([HD + 1, N], dt.float32, name="osb", tag="osb")
        nc.vector.tensor_copy(osb[:], ot_ps[:])
        rec = P["recp"].tile([1, N], dt.float32, name="rec", tag="rec")
        nc.vector.reciprocal(rec[0:1, :], osb[HD:HD + 1, :])
        brec = P["brec"].tile([HD, N], dt.float32, name="brec", tag="brec")
        nc.gpsimd.partition_broadcast(brec[0:HD, :], rec[0:1, :])
        nc.vector.tensor_mul(ot16[h // 2][ro:ro + HD, :],
                             osb[0:HD, :], brec[0:HD, :])
        yield


def _gen_d(nc, P, C, aps, b, st):
    """Output projection. Yields per t-chunk."""
    ot16 = st[f"ot{b}"]
    for (t0, p), ti in zip(T_CHUNKS, range(NT)):
        y_ps = P["ps_big"].tile([128, N], dt.float32, name="y_ps", tag="big")
        for (c0, cw) in COLS_D:
            for oi in range(ND):
                nc.tensor.matmul(
                    y_ps[0:p, c0:c0 + cw],
                    ot16[oi][:, t0:t0 + p],
                    C["wo16"][oi][:, c0:c0 + cw],
                    start=(oi == 0), stop=False)
            nc.tensor.matmul(
                y_ps[0:p, c0:c0 + cw],
                C["ones_c"][0:1, t0:t0 + p],
                C["bo16"][0:1, c0:c0 + cw],
                start=False, stop=True)
        y32 = P["yout"].tile([128, D], dt.float32, name="y32", tag="y32")
        nc.vector.tensor_copy(y32[0:p, :], y_ps[0:p, 0:D])
        nc.sync.dma_start(aps["ys"][b, t0:t0 + p, :], y32[0:p, :])
        yield


def _exhaust(g):
    for _ in g:
        pass


def _pull(g, k):
    """Pull up to k units from generator g; return #pulled."""
    n = 0
    for _ in range(k):
        try:
            next(g)
        except StopIteration:
            break
        n += 1
    return n


POOL_SPECS = [
    ("konst", 1, "SBUF"), ("wq", ND, "SBUF"), ("wo", ND, "SBUF"),
    ("xin", NT, "SBUF"), ("xt", ND, "SBUF"),
    ("qkt", 24, "SBUF"), ("vt", 2 * NT, "SBUF"), ("ex", 14, "SBUF"),
    ("ot", 2 * ND, "SBUF"), ("osb", 3, "SBUF"), ("brec", 3, "SBUF"), ("recp", 3, "SBUF"),
    ("yout", 2, "SBUF"),
    ("ps_big", 4, "PSUM"),
]


def build():
    nc = bacc.Bacc("TRN2", target_bir_lowering=False, debug=False)

    aps = {
        "xs": nc.dram_tensor("xs", [BPC, N, D], BF, kind="ExternalInput").ap(),
        "wqkv": nc.dram_tensor("wqkv", [D, F3], BF, kind="ExternalInput").ap(),
        "bqc": nc.dram_tensor("bqc", [128, F3 // 128], dt.float32, kind="ExternalInput").ap(),
        "bqv": nc.dram_tensor("bqv", [1, D], BF, kind="ExternalInput").ap(),
        "wo": nc.dram_tensor("wo", [D, D], BF, kind="ExternalInput").ap(),
        "bo": nc.dram_tensor("bo", [1, D], BF, kind="ExternalInput").ap(),
        "ys": nc.dram_tensor("ys", [BPC, N, D], dt.float32, kind="ExternalOutput").ap(),
    }

    with ExitStack() as ctx:
        tc = ctx.enter_context(tile.TileContext(nc))
        P = {}
        for pname, bufs, space in POOL_SPECS:
            P[pname] = ctx.enter_context(
                tc.tile_pool(name=pname, bufs=bufs, space=space))

        C = _setup_consts(nc, P, aps)
        st = {}
        # batch 0 prep
        _exhaust(_gen_a(nc, P, C, aps, 0, st))
        _exhaust(_gen_b_qk(nc, P, C, 0, st))
        _exhaust(_gen_b_v(nc, P, C, 0, st))
        # batch 0 attention, with batch 1 prep interleaved between heads
        import itertools
        prep1 = itertools.chain(
            _gen_a(nc, P, C, aps, 1, st),
            _gen_b_v(nc, P, C, 1, st),
            _gen_b_qk(nc, P, C, 1, st))
        c0 = _gen_c(nc, P, 0, st)
        for h in range(H):
            next(c0)
            _pull(prep1, 3 if h else 5)
        _exhaust(prep1)
        # batch 1 attention with batch 0 output projection interleaved
        c1 = _gen_c(nc, P, 1, st)
        d0 = _gen_d(nc, P, C, aps, 0, st)
        for h in range(H):
            next(c1)
            if h % 2 == 1:
                _pull(d0, 1)
        _exhaust(d0)
        _exhaust(_gen_d(nc, P, C, aps, 1, st))

    nc.compile()
    return nc


_NC_CACHE = {}


def _get_nc():
    if "nc" not in _NC_CACHE:
        _NC_CACHE["nc"] = build()
    return _NC_CACHE["nc"]


def make_in_maps(x, Wqkv, bqkv, Wo, bo):
    bf = ml_dtypes.bfloat16
    x = np.asarray(x, dtype=np.float32)
    Wqkv16 = np.ascontiguousarray(np.asarray(Wqkv, np.float32).astype(bf))
    bqkv = np.asarray(bqkv, dtype=np.float32)
    Wo16 = np.ascontiguousarray(np.asarray(Wo, np.float32).astype(bf))
    bo = np.asarray(bo, dtype=np.float32)
    bqc = np.ascontiguousarray(bqkv.reshape(F3 // 128, 128).T)
    bqv = np.ascontiguousarray(bqkv[2 * D:].reshape(1, D).astype(bf))
    bo_r = np.ascontiguousarray(bo.reshape(1, D).astype(bf))
    x16 = x.astype(bf)
    in_maps = []
    for c in range(N_CORES):
        in_maps.append({
            "xs": np.ascontiguousarray(x16[c * BPC:(c + 1) * BPC]),
            "wqkv": Wqkv16,
            "bqc": bqc,
            "bqv": bqv,
            "wo": Wo16,
            "bo": bo_r,
        })
    return in_maps


def run(x, Wqkv, bqkv, Wo, bo, trace=False, **kw):
    nc = _get_nc()
    in_maps = make_in_maps(x, Wqkv, bqkv, Wo, bo)
    res = run_bass_kernel_spmd(nc, in_maps, list(range(N_CORES)), trace=trace, **kw)
    out = np.concatenate([res.results[c]["ys"] for c in range(N_CORES)], axis=0)
    return out, res


def kernel(x, Wqkv, bqkv, Wo, bo):
    out, _ = run(x, Wqkv, bqkv, Wo, bo)
    return out


# revision 9
# speedup vs baseline: 1.3896x; 1.0159x over previous
"""Multi-head self-attention (B=16, N=784, D=768, H=12) on 8 trn2 cores.

Strategy: pure data-parallel over batch (2 batches per core, no collectives).
Host pre-casts x / weights / biases to bf16 (same rounding the device would
do); all matmuls run in bf16 with fp32 PSUM accumulation.

Per batch, on-device:
  A) X [784,768] is PE-transposed (bf16, transposes packed 4-per-PSUM-bank)
     into XT [768,784].
  B) QKV projection. Q,K are produced in transposed layout QKT [f, t]
     (stationary = Wqkv column block, moving = XT); the per-feature bias is
     added on DVE during the PSUM->SBUF copy. V is produced in natural
     layout [t, f] (stationary = XT chunk, moving = Wqkv V columns, bias via
     rank-1 accumulating matmul) and packed into an augmented slab
     [t, 12*(64+1)] whose per-head 65th column is 1.0.
  C) Per head: scores^T [tj, ti] = K^T-chunk.T @ Q^T (K=64 contraction);
     softmax without max-subtraction (scores are O(1) here): exp on ACT with
     the 1/8 scale fused in; PV with the ones-augmented V slab gives
     O^T[64,ti] plus the softmax denominator in row 64 for free. The PSUM
     result is copied to SBUF immediately (frees the bank), then normalized:
     in-place reciprocal of the denominator row (DVE), partition-broadcast
     (GpSimd), multiply (DVE).
  D) Output projection from OT (already the required lhsT layout) + bias via
     rank-1 accumulating matmuls.

The two batches are software-pipelined at emission level: batch 1's
transpose/projection units are interleaved between batch 0's attention
heads, and batch 0's output projection is interleaved into batch 1's
attention, so the PE never drains while ACT (softmax exp) streams.
"""

from contextlib import ExitStack

import ml_dtypes
import numpy as np

import concourse.bass as bass
import concourse.mybir as mybir
import concourse.tile as tile
from concourse import bacc
from concourse.bass_utils import run_bass_kernel_spmd
from concourse.masks import make_identity

dt = mybir.dt
AF = mybir.ActivationFunctionType

B, N, D = 16, 784, 768
H, HD = 12, 64
F3 = 3 * D  # 2304
N_CORES = 8
BPC = B // N_CORES  # batches per core

# token chunks: 784 = 6*128 + 16
T_CHUNKS = [(i * 128, min(128, N - i * 128)) for i in range((N + 127) // 128)]
NT = len(T_CHUNKS)  # 7
ND = D // 128  # 6 d-chunks
# transpose packing groups: 4+3 t-chunks -> one PSUM bank each
TR_GROUPS = [T_CHUNKS[0:4], T_CHUNKS[4:7]]
# psum column groups (bank-aligned: one fp32 bank holds 512)
COLS_N = [(0, 512), (512, N - 512)]   # over 784 tokens
COLS_D = [(0, 512), (512, D - 512)]   # over 768 features

BF = dt.bfloat16


def _setup_consts(nc, P, aps):
    ident = P["konst"].tile([128, 128], BF, name="ident")
    make_identity(nc, ident[:])
    ones_c = P["konst"].tile([1, N], BF, name="ones_c")
    nc.vector.memset(ones_c[:], 1.0)

    bqc = P["konst"].tile([128, F3 // 128], dt.float32, name="bqc")
    nc.sync.dma_start(bqc[:], aps["bqc"][:])
    bqv16 = P["konst"].tile([1, D], BF, name="bqv16")
    nc.sync.dma_start(bqv16[:], aps["bqv"][:])
    bo16 = P["konst"].tile([1, D], BF, name="bo16")
    nc.sync.dma_start(bo16[:], aps["bo"][:])

    wq16, wo16 = [], []
    for di in range(ND):
        w = P["wq"].tile([128, F3], BF, name=f"wq{di}", tag="wq")
        nc.sync.dma_start(w[:], aps["wqkv"][di * 128:(di + 1) * 128, :])
        wq16.append(w)
    for di in range(ND):
        w = P["wo"].tile([128, D], BF, name=f"wo{di}", tag="wo")
        nc.sync.dma_start(w[:], aps["wo"][di * 128:(di + 1) * 128, :])
        wo16.append(w)
    return dict(ident=ident, ones_c=ones_c, bqc=bqc, bqv16=bqv16, bo16=bo16,
                wq16=wq16, wo16=wo16)


def _gen_a(nc, P, C, aps, b, st):
    """Load X (bf16) and transpose to XT [768, 784]. Yields per unit."""
    x16 = []
    for (t0, p), ti in zip(T_CHUNKS, range(NT)):
        x = P["xin"].tile([128, D], BF, name=f"x16_{b}_{ti}", tag="x16")
        nc.sync.dma_start(x[0:p, :], aps["xs"][b, t0:t0 + p, :])
        x16.append(x)
    yield
    xt16 = [P["xt"].tile([128, N], BF, name=f"xt{b}_{di}", tag="xt")
            for di in range(ND)]
    for di in range(ND):
        for grp in TR_GROUPS:
            g0 = grp[0][0]
            gw = grp[-1][0] + grp[-1][1] - g0
            tr = P["ps_big"].tile([128, 512], BF, name="tr", tag="big")
            for (t0, p) in grp:
                nc.tensor.transpose(
                    tr[0:128, t0 - g0:t0 - g0 + p],
                    x16[t0 // 128][0:p, di * 128:(di + 1) * 128],
                    C["ident"][0:p, 0:p])
            nc.vector.tensor_copy(xt16[di][:, g0:g0 + gw], tr[0:128, 0:gw])
        yield
    st[f"xt{b}"] = xt16


def _gen_b_qk(nc, P, C, b, st):
    """Q,K in transposed layout: 12 tiles [128, 784]. Yields per f-chunk."""
    xt16 = st[f"xt{b}"]
    qkt16 = []
    st[f"qkt{b}"] = qkt16
    for fi in range(12):  # Q: 0..5, K: 6..11
        qk_ps = P["ps_big"].tile([128, N], dt.float32, name="qk_ps", tag="big")
        for (c0, cw) in COLS_N:
            for di in range(ND):
                nc.tensor.matmul(
                    qk_ps[:, c0:c0 + cw],
                    C["wq16"][di][:, fi * 128:(fi + 1) * 128],
                    xt16[di][:, c0:c0 + cw],
                    start=(di == 0), stop=(di == ND - 1))
        q = P["qkt"].tile([128, N], BF, name=f"qkt{b}_{fi}", tag="qkt")
        nc.vector.tensor_scalar_add(q[:], qk_ps[:], C["bqc"][0:128, fi:fi + 1])
        qkt16.append(q)
        yield


def _gen_b_v(nc, P, C, b, st):
    """V natural layout + ones column. Yields per t-chunk."""
    xt16 = st[f"xt{b}"]
    vt16 = []
    st[f"vt{b}"] = vt16
    for (t0, p), ti in zip(T_CHUNKS, range(NT)):
        v_ps = P["ps_big"].tile([128, N], dt.float32, name="v_ps", tag="big")
        for (c0, cw) in COLS_D:
            for di in range(ND):
                nc.tensor.matmul(
                    v_ps[0:p, c0:c0 + cw],
                    xt16[di][:, t0:t0 + p],
                    C["wq16"][di][:, 2 * D + c0:2 * D + c0 + cw],
                    start=(di == 0), stop=False)
            nc.tensor.matmul(
                v_ps[0:p, c0:c0 + cw],
                C["ones_c"][0:1, t0:t0 + p],
                C["bqv16"][0:1, c0:c0 + cw],
                start=False, stop=True)
        vt = P["vt"].tile([128, H, HD + 1], BF, name=f"vt{b}_{ti}", tag="vt")
        nc.vector.tensor_copy(vt[0:p, :, 0:HD],
                              v_ps[0:p, 0:D].rearrange("p (h d) -> p h d", h=H))
        nc.vector.memset(vt[0:p, :, HD:HD + 1], 1.0)
        vt16.append(vt)
        yield


def _gen_c(nc, P, b, st):
    """Attention. Yields per head."""
    qkt16, vt16 = st[f"qkt{b}"], st[f"vt{b}"]
    ot16 = [P["ot"].tile([128, N], BF, name=f"ot{b}_{oi}", tag="ot")
            for oi in range(ND)]
    st[f"ot{b}"] = ot16
    for h in range(H):
        qt, kt, ro = qkt16[h // 2], qkt16[6 + h // 2], (h % 2) * HD
        # scores^T + exp per tj chunk
        exl = []
        for (t0, pj), tj in zip(T_CHUNKS, range(NT)):
            sc_ps = P["ps_big"].tile([128, N], dt.float32, name="sc_ps", tag="big")
            for (c0, cw) in COLS_N:
                nc.tensor.matmul(
                    sc_ps[0:pj, c0:c0 + cw],
                    kt[ro:ro + HD, t0:t0 + pj],
                    qt[ro:ro + HD, c0:c0 + cw],
                    start=True, stop=True)
            ex = P["ex"].tile([128, N], BF, name="ex", tag="ex")
            nc.scalar.activation(ex[0:pj, :], sc_ps[0:pj, :], AF.Exp,
                                 scale=float(HD) ** -0.5)
            exl.append(ex)
        # PV with ones-augmented V: row 64 = softmax denominator
        ot_ps = P["ps_big"].tile([HD + 1, N], dt.float32, name="ot_ps", tag="big")
        for (c0, cw) in COLS_N:
            for (t0, pj), tj in zip(T_CHUNKS, range(NT)):
                nc.tensor.matmul(
                    ot_ps[0:HD + 1, c0:c0 + cw],
                    vt16[tj][0:pj, h, 0:HD + 1],
                    exl[tj][0:pj, c0:c0 + cw],
                    start=(tj == 0), stop=(tj == NT - 1))
        # copy out of PSUM immediately (frees the bank), then normalize
        osb = P["osb"].tile([HD + 1, N], dt.float32, name="osb", tag="osb")
        nc.vector.tensor_copy(osb[:], ot_ps[:])
        rec = P["recp"].tile([1, N], dt.float32, name="rec", tag="rec")
        nc.vector.reciprocal(rec[0:1, :], osb[HD:HD + 1, :])
        brec = P["brec"].tile([HD, N], dt.float32, name="brec", tag="brec")
        nc.gpsimd.partition_broadcast(brec[0:HD, :], rec[0:1, :])
        nc.vector.tensor_mul(ot16[h // 2][ro:ro + HD, :],
                             osb[0:HD, :], brec[0:HD, :])
        yield


def _gen_d(nc, P, C, aps, b, st):
    """Output projection. Yields per t-chunk."""
    ot16 = st[f"ot{b}"]
    for (t0, p), ti in zip(T_CHUNKS, range(NT)):
        y_ps = P["ps_big"].tile([128, N], dt.float32, name="y_ps", tag="big")
        for (c0, cw) in COLS_D:
            for oi in range(ND):
                nc.tensor.matmul(
                    y_ps[0:p, c0:c0 + cw],
                    ot16[oi][:, t0:t0 + p],
                    C["wo16"][oi][:, c0:c0 + cw],
                    start=(oi == 0), stop=False)
            nc.tensor.matmul(
                y_ps[0:p, c0:c0 + cw],
                C["ones_c"][0:1, t0:t0 + p],
                C["bo16"][0:1, c0:c0 + cw],
                start=False, stop=True)
        y32 = P["yout"].tile([128, D], dt.float32, name="y32", tag="y32")
        nc.vector.tensor_copy(y32[0:p, :], y_ps[0:p, 0:D])
        nc.sync.dma_start(aps["ys"][b, t0:t0 + p, :], y32[0:p, :])
        yield


def _exhaust(g):
    for _ in g:
        pass


def _pull(g, k):
    """Pull up to k units from generator g; return #pulled."""
    n = 0
    for _ in range(k):
        try:
            next(g)
        except StopIteration:
            break
        n += 1
    return n


POOL_SPECS = [
    ("konst", 1, "SBUF"), ("wq", ND, "SBUF"), ("wo", ND, "SBUF"),
    ("xin", NT, "SBUF"), ("xt", ND, "SBUF"),
    ("qkt", 24, "SBUF"), ("vt", 2 * NT, "SBUF"), ("ex", 16, "SBUF"),
    ("ot", 2 * ND, "SBUF"), ("osb", 3, "SBUF"), ("brec", 3, "SBUF"), ("recp", 3, "SBUF"),
    ("yout", 2, "SBUF"),
    ("ps_big", 4, "PSUM"),
]


def build():
    nc = bacc.Bacc("TRN2", target_bir_lowering=False, debug=False)

    aps = {
        "xs": nc.dram_tensor("xs", [BPC, N, D], BF, kind="ExternalInput").ap(),
        "wqkv": nc.dram_tensor("wqkv", [D, F3], BF, kind="ExternalInput").ap(),
        "bqc": nc.dram_tensor("bqc", [128, F3 // 128], dt.float32, kind="ExternalInput").ap(),
        "bqv": nc.dram_tensor("bqv", [1, D], BF, kind="ExternalInput").ap(),
        "wo": nc.dram_tensor("wo", [D, D], BF, kind="ExternalInput").ap(),
        "bo": nc.dram_tensor("bo", [1, D], BF, kind="ExternalInput").ap(),
        "ys": nc.dram_tensor("ys", [BPC, N, D], dt.float32, kind="ExternalOutput").ap(),
    }

    with ExitStack() as ctx:
        tc = ctx.enter_context(tile.TileContext(nc))
        P = {}
        for pname, bufs, space in POOL_SPECS:
            P[pname] = ctx.enter_context(
                tc.tile_pool(name=pname, bufs=bufs, space=space))

        C = _setup_consts(nc, P, aps)
        st = {}
        # batch 0 prep
        _exhaust(_gen_a(nc, P, C, aps, 0, st))
        _exhaust(_gen_b_qk(nc, P, C, 0, st))
        _exhaust(_gen_b_v(nc, P, C, 0, st))
        # batch 0 attention, with batch 1 prep interleaved between heads
        import itertools
        prep1 = itertools.chain(
            _gen_a(nc, P, C, aps, 1, st),
            _gen_b_v(nc, P, C, 1, st),
            _gen_b_qk(nc, P, C, 1, st))
        c0 = _gen_c(nc, P, 0, st)
        for h in range(H):
            next(c0)
            _pull(prep1, 2)
        _exhaust(prep1)
        # batch 1 attention with batch 0 output projection interleaved
        c1 = _gen_c(nc, P, 1, st)
        d0 = _gen_d(nc, P, C, aps, 0, st)
        for h in range(H):
            next(c1)
            if h % 2 == 1:
                _pull(d0, 1)
        _exhaust(d0)
        _exhaust(_gen_d(nc, P, C, aps, 1, st))

    nc.compile()
    return nc


_NC_CACHE = {}


def _get_nc():
    if "nc" not in _NC_CACHE:
        _NC_CACHE["nc"] = build()
    return _NC_CACHE["nc"]


def make_in_maps(x, Wqkv, bqkv, Wo, bo):
    bf = ml_dtypes.bfloat16
    x = np.asarray(x, dtype=np.float32)
    Wqkv16 = np.ascontiguousarray(np.asarray(Wqkv, np.float32).astype(bf))
    bqkv = np.asarray(bqkv, dtype=np.float32)
    Wo16 = np.ascontiguousarray(np.asarray(Wo, np.float32).astype(bf))
    bo = np.asarray(bo, dtype=np.float32)
    bqc = np.ascontiguousarray(bqkv.reshape(F3 // 128, 128).T)
    bqv = np.ascontiguousarray(bqkv[2 * D:].reshape(1, D).astype(bf))
    bo_r = np.ascontiguousarray(bo.reshape(1, D).astype(bf))
    x16 = x.astype(bf)
    in_maps = []
    for c in range(N_CORES):
        in_maps.append({
            "xs": np.ascontiguousarray(x16[c * BPC:(c + 1) * BPC]),
            "wqkv": Wqkv16,
            "bqc": bqc,
            "bqv": bqv,
            "wo": Wo16,
            "bo": bo_r,
        })
    return in_maps


def run(x, Wqkv, bqkv, Wo, bo, trace=False, **kw):
    nc = _get_nc()
    in_maps = make_in_maps(x, Wqkv, bqkv, Wo, bo)
    res = run_bass_kernel_spmd(nc, in_maps, list(range(N_CORES)), trace=trace, **kw)
    out = np.concatenate([res.results[c]["ys"] for c in range(N_CORES)], axis=0)
    return out, res


def kernel(x, Wqkv, bqkv, Wo, bo):
    out, _ = run(x, Wqkv, bqkv, Wo, bo)
    return out


# revision 14
# speedup vs baseline: 1.6714x; 1.2028x over previous
"""Multi-head self-attention (B=16, N=784, D=768, H=12) on 8 trn2 cores.

Strategy: pure data-parallel over batch (2 batches per core, no collectives).
Host pre-casts x / weights / biases to bf16 (same rounding the device would
do); all matmuls run in bf16 with fp32 PSUM accumulation.

Per batch, on-device:
  A) X [784,768] is PE-transposed (bf16, transposes packed 4-per-PSUM-bank)
     into XT [768,784].
  B) QKV projection. Q,K are produced in transposed layout QKT [f, t]
     (stationary = Wqkv column block, moving = XT); the per-feature bias is
     added on DVE during the PSUM->SBUF copy. V is produced in natural
     layout [t, f] (stationary = XT chunk, moving = Wqkv V columns, bias via
     rank-1 accumulating matmul) and packed into an augmented slab
     [t, 12*(64+1)] whose per-head 65th column is 1.0.
  C) Per head: scores^T [tj, ti] = K^T-chunk.T @ Q^T (K=64 contraction);
     softmax without max-subtraction (scores are O(1) here): exp on ACT with
     the 1/8 scale fused in; PV with the ones-augmented V slab gives
     O^T[64,ti] plus the softmax denominator in row 64 for free. The PSUM
     result is copied to SBUF immediately (frees the bank), then normalized:
     in-place reciprocal of the denominator row (DVE), partition-broadcast
     (GpSimd), multiply (DVE).
  D) Output projection from OT (already the required lhsT layout) + bias via
     rank-1 accumulating matmuls.

The two batches are software-pipelined at emission level: batch 1's
transpose/projection units are interleaved between batch 0's attention
heads, and batch 0's output projection is interleaved into batch 1's
attention, so the PE never drains while ACT (softmax exp) streams.
"""

from contextlib import ExitStack

import ml_dtypes
import numpy as np

import concourse.bass as bass
import concourse.mybir as mybir
import concourse.tile as tile
from concourse import bacc
from concourse.bass_utils import run_bass_kernel_spmd
from concourse.masks import make_identity

dt = mybir.dt
AF = mybir.ActivationFunctionType

B, N, D = 16, 784, 768
H, HD = 12, 64
F3 = 3 * D  # 2304
N_CORES = 8
BPC = B // N_CORES  # batches per core

# token chunks: 784 = 6*128 + 16
T_CHUNKS = [(i * 128, min(128, N - i * 128)) for i in range((N + 127) // 128)]
NT = len(T_CHUNKS)  # 7
ND = D // 128  # 6 d-chunks
# transpose packing groups: 4+3 t-chunks -> one PSUM bank each
TR_GROUPS = [T_CHUNKS[0:4], T_CHUNKS[4:7]]
# psum column groups (bank-aligned: one fp32 bank holds 512)
COLS_N = [(0, 512), (512, N - 512)]   # over 784 tokens
COLS_D = [(0, 512), (512, D - 512)]   # over 768 features

BF = dt.bfloat16


def _setup_consts(nc, P, aps):
    ident = P["konst"].tile([128, 128], BF, name="ident")
    make_identity(nc, ident[:])
    ones_c = P["konst"].tile([1, N], BF, name="ones_c")
    nc.vector.memset(ones_c[:], 1.0)

    bqc = P["konst"].tile([128, F3 // 128], dt.float32, name="bqc")
    nc.sync.dma_start(bqc[:], aps["bqc"][:])
    bqv16 = P["konst"].tile([1, D], BF, name="bqv16")
    nc.sync.dma_start(bqv16[:], aps["bqv"][:])
    bo16 = P["konst"].tile([1, D], BF, name="bo16")
    nc.sync.dma_start(bo16[:], aps["bo"][:])

    wq16, wo16 = [], []
    for di in range(ND):
        w = P["wq"].tile([128, F3], BF, name=f"wq{di}", tag="wq")
        nc.sync.dma_start(w[:], aps["wqkv"][di * 128:(di + 1) * 128, :])
        wq16.append(w)
    for di in range(ND):
        w = P["wo"].tile([128, D], BF, name=f"wo{di}", tag="wo")
        nc.sync.dma_start(w[:], aps["wo"][di * 128:(di + 1) * 128, :])
        wo16.append(w)
    return dict(ident=ident, ones_c=ones_c, bqc=bqc, bqv16=bqv16, bo16=bo16,
                wq16=wq16, wo16=wo16)


def _gen_a(nc, P, C, aps, b, st):
    """Load X (bf16) and transpose to XT [768, 784]. Yields per unit."""
    x16 = []
    for (t0, p), ti in zip(T_CHUNKS, range(NT)):
        x = P["xin"].tile([128, D], BF, name=f"x16_{b}_{ti}", tag="x16")
        nc.sync.dma_start(x[0:p, :], aps["xs"][b, t0:t0 + p, :])
        x16.append(x)
    yield
    xt16 = [P["xt"].tile([128, N], BF, name=f"xt{b}_{di}", tag="xt")
            for di in range(ND)]
    for di in range(ND):
        for grp in TR_GROUPS:
            g0 = grp[0][0]
            gw = grp[-1][0] + grp[-1][1] - g0
            tr = P["ps_big"].tile([128, 512], BF, name="tr", tag="big")
            for (t0, p) in grp:
                nc.tensor.transpose(
                    tr[0:128, t0 - g0:t0 - g0 + p],
                    x16[t0 // 128][0:p, di * 128:(di + 1) * 128],
                    C["ident"][0:p, 0:p])
            nc.vector.tensor_copy(xt16[di][:, g0:g0 + gw], tr[0:128, 0:gw])
        yield
    st[f"xt{b}"] = xt16


def _gen_b_qk(nc, P, C, b, st):
    """Q,K in transposed layout: 12 tiles [128, 784]. Yields per f-chunk."""
    xt16 = st[f"xt{b}"]
    qkt16 = []
    st[f"qkt{b}"] = qkt16
    for fi in range(12):  # Q: 0..5, K: 6..11
        qk_ps = P["ps_big"].tile([128, N], dt.float32, name="qk_ps", tag="big")
        for (c0, cw) in COLS_N:
            for di in range(ND):
                nc.tensor.matmul(
                    qk_ps[:, c0:c0 + cw],
                    C["wq16"][di][:, fi * 128:(fi + 1) * 128],
                    xt16[di][:, c0:c0 + cw],
                    start=(di == 0), stop=(di == ND - 1))
        q = P["qkt"].tile([128, N], BF, name=f"qkt{b}_{fi}", tag="qkt")
        nc.vector.tensor_scalar_add(q[:], qk_ps[:], C["bqc"][0:128, fi:fi + 1])
        qkt16.append(q)
        yield


def _gen_b_v(nc, P, C, b, st):
    """V natural layout + ones column. Yields per t-chunk."""
    xt16 = st[f"xt{b}"]
    vt16 = []
    st[f"vt{b}"] = vt16
    for (t0, p), ti in zip(T_CHUNKS, range(NT)):
        v_ps = P["ps_big"].tile([128, N], dt.float32, name="v_ps", tag="big")
        for (c0, cw) in COLS_D:
            for di in range(ND):
                nc.tensor.matmul(
                    v_ps[0:p, c0:c0 + cw],
                    xt16[di][:, t0:t0 + p],
                    C["wq16"][di][:, 2 * D + c0:2 * D + c0 + cw],
                    start=(di == 0), stop=False)
            nc.tensor.matmul(
                v_ps[0:p, c0:c0 + cw],
                C["ones_c"][0:1, t0:t0 + p],
                C["bqv16"][0:1, c0:c0 + cw],
                start=False, stop=True)
        vt = P["vt"].tile([128, H, HD + 1], BF, name=f"vt{b}_{ti}", tag="vt")
        nc.vector.tensor_copy(vt[0:p, :, 0:HD],
                              v_ps[0:p, 0:D].rearrange("p (h d) -> p h d", h=H))
        nc.vector.memset(vt[0:p, :, HD:HD + 1], 1.0)
        vt16.append(vt)
        yield


def _gen_c(nc, P, b, st):
    """Attention. Yields per head."""
    qkt16, vt16 = st[f"qkt{b}"], st[f"vt{b}"]
    ot16 = [P["ot"].tile([128, N], BF, name=f"ot{b}_{oi}", tag="ot")
            for oi in range(ND)]
    st[f"ot{b}"] = ot16
    for h in range(H):
        qt, kt, ro = qkt16[h // 2], qkt16[6 + h // 2], (h % 2) * HD
        # scores^T + exp per tj chunk
        exl = []
        for (t0, pj), tj in zip(T_CHUNKS, range(NT)):
            sc_ps = P["ps_big"].tile([128, N], dt.float32, name="sc_ps", tag="big")
            for (c0, cw) in COLS_N:
                nc.tensor.matmul(
                    sc_ps[0:pj, c0:c0 + cw],
                    kt[ro:ro + HD, t0:t0 + pj],
                    qt[ro:ro + HD, c0:c0 + cw],
                    start=True, stop=True)
            ex = P["ex"].tile([128, N], BF, name="ex", tag="ex")
            nc.scalar.activation(ex[0:pj, :], sc_ps[0:pj, :], AF.Exp,
                                 scale=float(HD) ** -0.5)
            exl.append(ex)
        # PV with ones-augmented V: row 64 = softmax denominator
        ot_ps = P["ps_big"].tile([HD + 1, N], dt.float32, name="ot_ps", tag="big")
        for (c0, cw) in COLS_N:
            for (t0, pj), tj in zip(T_CHUNKS, range(NT)):
                nc.tensor.matmul(
                    ot_ps[0:HD + 1, c0:c0 + cw],
                    vt16[tj][0:pj, h, 0:HD + 1],
                    exl[tj][0:pj, c0:c0 + cw],
                    start=(tj == 0), stop=(tj == NT - 1))
        # copy out of PSUM immediately (frees the bank), then normalize;
        # the sums row is copied to partition 0 of its own tile
        # (reciprocal_approx_fast mis-handles partition-offset inputs)
        osb = P["osb"].tile([HD, N], dt.float32, name="osb", tag="osb")
        nc.vector.tensor_copy(osb[:], ot_ps[0:HD, :])
        srow = P["recp"].tile([1, N], dt.float32, name="srow", tag="srow")
        nc.vector.tensor_copy(srow[0:1, :], ot_ps[HD:HD + 1, :])
        rec = P["recp"].tile([1, N], dt.float32, name="rec", tag="rec")
        nc.vector.reciprocal_approx_fast(rec[0:1, :], srow[0:1, :])
        brec = P["brec"].tile([HD, N], dt.float32, name="brec", tag="brec")
        nc.gpsimd.partition_broadcast(brec[0:HD, :], rec[0:1, :])
        nc.vector.tensor_mul(ot16[h // 2][ro:ro + HD, :],
                             osb[0:HD, :], brec[0:HD, :])
        yield


def _gen_d(nc, P, C, aps, b, st):
    """Output projection. Yields per t-chunk."""
    ot16 = st[f"ot{b}"]
    for (t0, p), ti in zip(T_CHUNKS, range(NT)):
        y_ps = P["ps_big"].tile([128, N], dt.float32, name="y_ps", tag="big")
        for (c0, cw) in COLS_D:
            for oi in range(ND):
                nc.tensor.matmul(
                    y_ps[0:p, c0:c0 + cw],
                    ot16[oi][:, t0:t0 + p],
                    C["wo16"][oi][:, c0:c0 + cw],
                    start=(oi == 0), stop=False)
            nc.tensor.matmul(
                y_ps[0:p, c0:c0 + cw],
                C["ones_c"][0:1, t0:t0 + p],
                C["bo16"][0:1, c0:c0 + cw],
                start=False, stop=True)
        y32 = P["yout"].tile([128, D], dt.float32, name="y32", tag="y32")
        nc.vector.tensor_copy(y32[0:p, :], y_ps[0:p, 0:D])
        nc.sync.dma_start(aps["ys"][b, t0:t0 + p, :], y32[0:p, :])
        yield


def _exhaust(g):
    for _ in g:
        pass


def _pull(g, k):
    """Pull up to k units from generator g; return #pulled."""
    n = 0
    for _ in range(k):
        try:
            next(g)
        except StopIteration:
            break
        n += 1
    return n


POOL_SPECS = [
    ("konst", 1, "SBUF"), ("wq", ND, "SBUF"), ("wo", ND, "SBUF"),
    ("xin", NT, "SBUF"), ("xt", ND, "SBUF"),
    ("qkt", 24, "SBUF"), ("vt", 2 * NT, "SBUF"), ("ex", 16, "SBUF"),
    ("ot", 2 * ND, "SBUF"), ("osb", 3, "SBUF"), ("brec", 3, "SBUF"), ("recp", 3, "SBUF"),
    ("yout", 2, "SBUF"),
    ("ps_big", 4, "PSUM"),
]


def build():
    nc = bacc.Bacc("TRN2", target_bir_lowering=False, debug=False)

    aps = {
        "xs": nc.dram_tensor("xs", [BPC, N, D], BF, kind="ExternalInput").ap(),
        "wqkv": nc.dram_tensor("wqkv", [D, F3], BF, kind="ExternalInput").ap(),
        "bqc": nc.dram_tensor("bqc", [128, F3 // 128], dt.float32, kind="ExternalInput").ap(),
        "bqv": nc.dram_tensor("bqv", [1, D], BF, kind="ExternalInput").ap(),
        "wo": nc.dram_tensor("wo", [D, D], BF, kind="ExternalInput").ap(),
        "bo": nc.dram_tensor("bo", [1, D], BF, kind="ExternalInput").ap(),
        "ys": nc.dram_tensor("ys", [BPC, N, D], dt.float32, kind="ExternalOutput").ap(),
    }

    with ExitStack() as ctx:
        tc = ctx.enter_context(tile.TileContext(nc))
        P = {}
        for pname, bufs, space in POOL_SPECS:
            P[pname] = ctx.enter_context(
                tc.tile_pool(name=pname, bufs=bufs, space=space))

        C = _setup_consts(nc, P, aps)
        st = {}
        # batch 0 prep
        _exhaust(_gen_a(nc, P, C, aps, 0, st))
        _exhaust(_gen_b_qk(nc, P, C, 0, st))
        _exhaust(_gen_b_v(nc, P, C, 0, st))
        # batch 0 attention, with batch 1 prep interleaved between heads
        import itertools
        prep1 = itertools.chain(
            _gen_a(nc, P, C, aps, 1, st),
            _gen_b_v(nc, P, C, 1, st),
            _gen_b_qk(nc, P, C, 1, st))
        c0 = _gen_c(nc, P, 0, st)
        for h in range(H):
            next(c0)
            _pull(prep1, 2)
        _exhaust(prep1)
        # batch 1 attention with batch 0 output projection interleaved
        c1 = _gen_c(nc, P, 1, st)
        d0 = _gen_d(nc, P, C, aps, 0, st)
        for h in range(H):
            next(c1)
            if h % 2 == 1:
                _pull(d0, 1)
        _exhaust(d0)
        _exhaust(_gen_d(nc, P, C, aps, 1, st))

    nc.compile()
    return nc


_NC_CACHE = {}


def _get_nc():
    if "nc" not in _NC_CACHE:
        _NC_CACHE["nc"] = build()
    return _NC_CACHE["nc"]


def make_in_maps(x, Wqkv, bqkv, Wo, bo):
    bf = ml_dtypes.bfloat16
    x = np.asarray(x, dtype=np.float32)
    Wqkv16 = np.ascontiguousarray(np.asarray(Wqkv, np.float32).astype(bf))
    bqkv = np.asarray(bqkv, dtype=np.float32)
    Wo16 = np.ascontiguousarray(np.asarray(Wo, np.float32).astype(bf))
    bo = np.asarray(bo, dtype=np.float32)
    bqc = np.ascontiguousarray(bqkv.reshape(F3 // 128, 128).T)
    bqv = np.ascontiguousarray(bqkv[2 * D:].reshape(1, D).astype(bf))
    bo_r = np.ascontiguousarray(bo.reshape(1, D).astype(bf))
    x16 = x.astype(bf)
    in_maps = []
    for c in range(N_CORES):
        in_maps.append({
            "xs": np.ascontiguousarray(x16[c * BPC:(c + 1) * BPC]),
            "wqkv": Wqkv16,
            "bqc": bqc,
            "bqv": bqv,
            "wo": Wo16,
            "bo": bo_r,
        })
    return in_maps


def run(x, Wqkv, bqkv, Wo, bo, trace=False, **kw):
    nc = _get_nc()
    in_maps = make_in_maps(x, Wqkv, bqkv, Wo, bo)
    res = run_bass_kernel_spmd(nc, in_maps, list(range(N_CORES)), trace=trace, **kw)
    out = np.concatenate([res.results[c]["ys"] for c in range(N_CORES)], axis=0)
    return out, res


def kernel(x, Wqkv, bqkv, Wo, bo):
    out, _ = run(x, Wqkv, bqkv, Wo, bo)
    return out
